# revision 35
# baseline (speedup 1.0000x reference)
"""Trainium2 Bass kernel for ActionRecognitionCRF (emission GEMM + Viterbi decode).

Strategy (8 NeuronCores, no collectives):
  - Each core gets an overlapping video shard (its 2048 rows + the previous
    L0=128 rows) transposed on host to [K, rows], computes its feats window
    with the TensorEngine (weights stationary, video streaming, bias via a
    K=1 matmul, ReLU on the Scalar engine) directly in transposed [P, t]
    layout.
  - The Viterbi forward scan is chunk-parallel: each core runs 4 sub-chunk
    scans simultaneously on 4x32 lane groups (states packed 2-per-lane so the
    cross-state max stays inside 32-lane blocks where the DVE stream-transpose
    works).  Sub-chunks (except the global first) start from a host-calibrated
    magnitude-matched uniform init and burn in for L0 steps; max-plus
    coalescence plus grid alignment make the post-burn-in backpointers exact.
  - Backpointers are recomputed from the stored forward vectors in a second,
    fully parallel pass (PE transposes fv history and the scan's max history
    to [t, p] tiles; DVE add + max_index extract first-occurrence argmaxes,
    matching jnp.argmax tie semantics).
  - Host stitches backpointers, runs the O(T) backtrace and recomputes the
    path score along the decoded path in reference op order.
"""

import numpy as np

import concourse.bass as bass
import concourse.tile as tile
from concourse import bacc, mybir
from concourse.bass_utils import run_bass_kernel_spmd
from concourse.masks import make_identity

dt = mybir.dt

P = 64
START = P - 2
STOP = P - 1
NEG = np.float32(-10000.0)

CFG = dict(T=16384, K=4096, NCORES=8, NCH=4, CSZ=512, L0=32)

LAST_EXEC_TIME_NS = None


def _derived(cfg):
    d = dict(cfg)
    d["W"] = cfg["CSZ"] + cfg["L0"]          # scan window length per sub-chunk
    d["TLOC"] = cfg["NCH"] * cfg["CSZ"]      # real steps per core
    d["ROWS"] = d["TLOC"] + cfg["L0"]        # video rows per core
    d["LANES"] = 32 * cfg["NCH"]
    d["KB"] = cfg["K"] // 128
    return d


def build_graph(cfg=CFG):
    d = _derived(cfg)
    K, W, ROWS, LANES, KB, NCH, CSZ, L0 = (
        d["K"], d["W"], d["ROWS"], d["LANES"], d["KB"], d["NCH"], d["CSZ"], d["L0"]
    )
    nc = bacc.Bacc(
        "TRN2", target_bir_lowering=False, debug=False, num_devices=cfg["NCORES"]
    )
    videot_d = nc.dram_tensor("videot", [K, ROWS], dt.float32, kind="ExternalInput").ap()
    wt_d = nc.dram_tensor("wt", [K, P], dt.float32, kind="ExternalInput").ap()
    bias_d = nc.dram_tensor("bias", [1, P], dt.float32, kind="ExternalInput").ap()
    transt_d = nc.dram_tensor("transt", [P, P], dt.float32, kind="ExternalInput").ap()
    transf_d = nc.dram_tensor("transf", [1, P * P], dt.float32, kind="ExternalInput").ap()
    initv_d = nc.dram_tensor("initv", [LANES, 2], dt.float32, kind="ExternalInput").ap()
    bptrs_d = nc.dram_tensor("bptrs", [NCH * W, P], dt.uint8, kind="ExternalOutput").ap()
    fvout_d = nc.dram_tensor("fvout", [LANES, 2], dt.float32, kind="ExternalOutput").ap()
    featst_d = nc.dram_tensor("featst", [P, ROWS], dt.float32, kind="ExternalOutput").ap()

    with tile.TileContext(nc) as tc:
        with (
            tc.tile_pool(name="persist", bufs=1) as pp,
            tc.tile_pool(name="vt", bufs=3) as vpool,
            tc.tile_pool(name="vt0", bufs=4) as vpool0,
            tc.tile_pool(name="psum_r", bufs=1, space="PSUM") as psfr,
            tc.tile_pool(name="scan", bufs=2) as spool,
            tc.tile_pool(name="bptr", bufs=2) as bpool,
            tc.tile_pool(name="psum_f", bufs=1, space="PSUM") as psf,
            tc.tile_pool(name="psum_t", bufs=1, space="PSUM") as pst,
        ):
            # ---- static setup ----
            wts = pp.tile([128, KB, P], dt.float32)
            nc.sync.dma_start(wts[:], wt_d.rearrange("(kb r) p -> r kb p", r=128))
            bias_sb = pp.tile([1, P], dt.float32)
            nc.sync.dma_start(bias_sb[:], bias_d[:])
            ones = pp.tile([1, 512], dt.float32)
            nc.vector.memset(ones[:], 1.0)
            ident = pp.tile([128, 32], dt.float32)
            make_identity(nc, ident[0:32, :])
            for j in range(1, 4):
                nc.sync.dma_start(ident[j * 32:(j + 1) * 32, :], ident[0:32, :])

            transT = pp.tile([LANES, 2, P], dt.float32)
            for j in range(NCH):
                nc.sync.dma_start(
                    transT[j * 32:(j + 1) * 32, :, :],
                    transt_d.rearrange("(pb i) n -> i pb n", i=32),
                )
            transBC = pp.tile([128, P, P], dt.float32)
            nc.sync.dma_start(
                transBC.rearrange("r n p -> r (n p)"),
                transf_d[:].broadcast_to([128, P * P]),
            )

            M = pp.tile([LANES, 2, W + 1], dt.float32)
            nc.sync.dma_start(M[:, :, 0], initv_d[:])
            FEAT2 = pp.tile([LANES, 2, W + 1], dt.float32)
            nc.vector.memset(FEAT2[:, :, 0], 0.0)
            FVH = pp.tile([LANES, 2, W + 1], dt.float32)
            # feats in per-block tiles, GEMM'd in window-interleaved order so
            # the scan (which consumes one column of every window per step)
            # can start after the first round of blocks instead of after the
            # whole GEMM
            GB = min(256, CSZ)
            fb_sizes = []
            for s0 in range(0, CSZ, GB):
                for j in range(NCH):
                    fb_sizes.append((j * CSZ + s0, min(GB, ROWS - (j * CSZ + s0))))
            t0 = NCH * CSZ
            while t0 < ROWS:
                fb_sizes.append((t0, min(GB, ROWS - t0)))
                t0 += min(GB, ROWS - t0)
            fblocks = [
                pp.tile([P, nt], dt.float32, tag=f"fb{i}", name=f"fb{i}")
                for i, (_, nt) in enumerate(fb_sizes)
            ]

            # all (window, half, segment) fills keyed by the global col range
            segs = []
            for j in range(NCH):
                for b in range(2):
                    s0 = 0
                    while s0 < W:
                        ns = min(128, W - s0)
                        segs.append((j, b, s0, ns))
                        s0 += ns

            # ---- GEMM: feats[p, t] = relu(video @ W.T + b).T ----
            # Each round loads all NCH window blocks' k-slices in ONE strided
            # DMA per k-chunk (sequencer dispatch, not DMA bandwidth, limits
            # the pre-scan round) and keeps each weight chunk loaded for all
            # NCH matmuls.  Window fills are emitted right after the block
            # that produces their data.
            def emit_fills(i, t0, nt):
                for (j, b, s0, ns) in segs:
                    lo = j * CSZ + s0
                    if t0 <= lo and lo + ns <= t0 + nt:
                        nc.sync.dma_start(
                            FEAT2[j * 32:(j + 1) * 32, b, 1 + s0:1 + s0 + ns],
                            fblocks[i][32 * b:32 * b + 32, lo - t0:lo - t0 + ns],
                        )

            n_rounds = CSZ // GB
            for r in range(n_rounds):
                psums = [
                    psfr.tile([P, GB], dt.float32, name=f"ps{j}") for j in range(NCH)
                ]
                vp = vpool0 if r == 0 else vpool
                for kb in range(KB):
                    vt = vp.tile([128, NCH, GB], dt.float32,
                                 name="vt0" if r == 0 else "vt")
                    nc.sync.dma_start(
                        vt[:],
                        videot_d[kb * 128:(kb + 1) * 128, 0:NCH * CSZ].rearrange(
                            "r (j t) -> r j t", t=CSZ
                        )[:, :, r * GB:r * GB + GB],
                    )
                    for j in range(NCH):
                        nc.tensor.matmul(
                            psums[j][:], wts[:, kb, :], vt[:, j, :],
                            start=(kb == 0), stop=False, skip_group_check=True,
                        )
                for j in range(NCH):
                    nc.tensor.matmul(
                        psums[j][:], bias_sb[:], ones[:, 0:GB],
                        start=False, stop=True, skip_group_check=True,
                    )
                    i = r * NCH + j
                    nc.scalar.activation(
                        fblocks[i][:], psums[j][:],
                        mybir.ActivationFunctionType.Relu,
                    )
                    emit_fills(i, fb_sizes[i][0], fb_sizes[i][1])
            for i in range(n_rounds * NCH, len(fb_sizes)):
                t0, nt = fb_sizes[i]
                psum = psf.tile([P, 512], dt.float32)
                for kb in range(KB):
                    vt = vpool.tile([128, NCH, GB], dt.float32, name="vt")
                    nc.sync.dma_start(
                        vt[:, 0, 0:nt],
                        videot_d[kb * 128:(kb + 1) * 128, t0:t0 + nt],
                    )
                    nc.tensor.matmul(
                        psum[:, 0:nt], wts[:, kb, :], vt[:, 0, 0:nt],
                        start=(kb == 0), stop=False,
                    )
                nc.tensor.matmul(
                    psum[:, 0:nt], bias_sb[:], ones[:, 0:nt], start=False, stop=True
                )
                nc.scalar.activation(
                    fblocks[i][:], psum[:, 0:nt],
                    mybir.ActivationFunctionType.Relu,
                )
                emit_fills(i, t0, nt)

            # ---- Viterbi scan (NCH sub-chunks in parallel on lane groups) ----
            for w in range(W + 1):
                nc.vector.tensor_add(FVH[:, :, w], M[:, :, w], FEAT2[:, :, w])
                if w == W:
                    break
                st = spool.tile([LANES, 2, P], dt.float32)
                nc.vector.tensor_add(
                    st[:], transT[:], FVH[:, :, w:w + 1].broadcast_to([LANES, 2, P])
                )
                x = spool.tile([LANES, 2, 2, 32], dt.float32)  # (pb, nb, k)
                nc.vector.transpose(
                    x.rearrange("p pb nb k -> p (pb nb k)"),
                    st.rearrange("p pb n -> p (pb n)"),
                )
                nc.vector.tensor_reduce(
                    M[:, :, w + 1],
                    x.rearrange("p pb nb k -> p nb pb k"),
                    axis=mybir.AxisListType.XY,
                    op=mybir.AluOpType.max,
                )
            nc.sync.dma_start(fvout_d[:], FVH[:, :, W])

            # ---- backpointer recompute pass ----
            # Reuses the scan's max values (M history) so only the index
            # search and the batched adds run on DVE.
            NB = 8
            for j in range(NCH):
                w0 = 0 if j == 0 else L0
                while w0 < W:
                    nr = min(128, W - w0)
                    pfv = pst.tile([128, P], dt.float32)
                    pmx = pst.tile([128, P], dt.float32)
                    for b in range(2):
                        nc.tensor.transpose(
                            pfv[0:nr, b * 32:(b + 1) * 32],
                            FVH[j * 32:(j + 1) * 32, b, w0:w0 + nr],
                            ident[j * 32:(j + 1) * 32, :],
                            tile_position=(j * 32, 0),
                        )
                        nc.tensor.transpose(
                            pmx[0:nr, b * 32:(b + 1) * 32],
                            M[j * 32:(j + 1) * 32, b, w0 + 1:w0 + 1 + nr],
                            ident[j * 32:(j + 1) * 32, :],
                            tile_position=(j * 32, 0),
                        )
                    fv_sb = bpool.tile([128, P], dt.float32)
                    nc.scalar.activation(
                        fv_sb[0:nr, :], pfv[0:nr, :],
                        mybir.ActivationFunctionType.Copy,
                    )
                    mx_sb = bpool.tile([128, P], dt.float32)
                    nc.scalar.activation(
                        mx_sb[0:nr, :], pmx[0:nr, :],
                        mybir.ActivationFunctionType.Copy,
                    )
                    bp8 = bpool.tile([128, P, 8], dt.uint32)
                    for n0 in range(0, P, NB):
                        tmp = bpool.tile([128, NB, P], dt.float32)
                        nc.vector.tensor_add(
                            tmp[0:nr, :, :],
                            transBC[0:nr, n0:n0 + NB, :],
                            fv_sb[0:nr, :].rearrange(
                                "t (o p) -> t o p", o=1
                            ).broadcast_to([nr, NB, P]),
                        )
                        for k in range(NB):
                            nc.vector.max_index(
                                bp8[0:nr, n0 + k, :],
                                mx_sb[0:nr, n0 + k:n0 + k + 1].broadcast_to([nr, 8]),
                                tmp[0:nr, k, :],
                            )
                    bpu8 = bpool.tile([128, P], dt.uint8)
                    nc.vector.tensor_copy(bpu8[0:nr, :], bp8[0:nr, :, 0])
                    nc.sync.dma_start(
                        bptrs_d[j * W + w0:j * W + w0 + nr, :], bpu8[0:nr, :]
                    )
                    w0 += nr

            for (t0, nt), fb in zip(fb_sizes, fblocks):
                nc.sync.dma_start(featst_d[:, t0:t0 + nt], fb[:])

    nc.compile()
    return nc


_NC_CACHE = {}


def _get_nc(cfg_key):
    if cfg_key not in _NC_CACHE:
        _NC_CACHE[cfg_key] = build_graph(CFG)
    return _NC_CACHE[cfg_key]


def _scan_fv(feats_win, fv, trans):
    fv = fv.copy()
    hist = np.empty((feats_win.shape[0], P), np.float32)
    for t in range(feats_win.shape[0]):
        scores = (fv[None, :] + trans).astype(np.float32)
        fv = (scores.max(axis=1) + feats_win[t]).astype(np.float32)
        hist[t] = fv
    return hist


def _calibrate(video, Wm, bvec, trans, cfg):
    """Estimate the forward-vector magnitude at each sub-chunk start via an
    anchored mini-scan + sampled slopes (sets only scan init constants)."""
    T = cfg["T"]
    fv_init = np.full(P, NEG, np.float32)
    fv_init[START] = 0.0

    def host_feats(sl):
        return np.maximum(
            (video[sl].astype(np.float32) @ Wm.T + bvec).astype(np.float32), 0.0
        )

    h0 = _scan_fv(host_feats(slice(0, 384)), fv_init, trans)
    phi0 = h0.max(axis=1)
    anchor_v = phi0[-1]
    slopes = {384: (phi0[383] - phi0[255]) / 128.0}
    n_anchor = max(1, T // 2048)
    anchor_ts = [384]
    for m in range(1, n_anchor):
        tm = 2048 * m
        hw = _scan_fv(
            host_feats(slice(tm - 96, tm + 160)), np.zeros(P, np.float32), trans
        )
        phi = hw.max(axis=1)
        slopes[tm] = (phi[255] - phi[127]) / 128.0
        anchor_ts.append(tm)

    def vhat(s):
        if s <= 384:
            return anchor_v + slopes[384] * (s - 384)
        v = anchor_v
        prev = 384
        for a in anchor_ts[1:] + [T]:
            seg_end = min(a, s)
            sl_lo = slopes[prev] if prev in slopes else slopes[384]
            sl_hi = slopes[a] if a in slopes else slopes[anchor_ts[-1]]
            v += (sl_lo + sl_hi) / 2 * (seg_end - prev)
            prev = seg_end
            if seg_end >= s:
                break
        return v

    return vhat


def kernel(video, W, b, transitions, _trace=False):
    global LAST_EXEC_TIME_NS
    cfg = CFG
    d = _derived(cfg)
    T, K, NCORES, NCH, CSZ, L0 = (
        cfg["T"], cfg["K"], cfg["NCORES"], cfg["NCH"], cfg["CSZ"], cfg["L0"]
    )
    Wn, TLOC, ROWS, LANES = d["W"], d["TLOC"], d["ROWS"], d["LANES"]

    video = np.ascontiguousarray(np.asarray(video, np.float32))
    Wm = np.ascontiguousarray(np.asarray(W, np.float32))
    bvec = np.asarray(b, np.float32).reshape(-1)
    trans = np.ascontiguousarray(np.asarray(transitions, np.float32))

    fv_init = np.full(P, NEG, np.float32)
    fv_init[START] = 0.0

    vhat = _calibrate(video, Wm, bvec, trans, cfg)

    videot = np.ascontiguousarray(video.T)  # [K, T]
    wt = np.ascontiguousarray(Wm.T)         # [K, P]
    transt = np.ascontiguousarray(trans.T)
    transf = np.ascontiguousarray(trans.reshape(1, P * P))

    in_maps = []
    for c in range(NCORES):
        a = 0 if c == 0 else c * TLOC - L0
        shard = np.ascontiguousarray(videot[:, a:a + ROWS])
        initv = np.empty((LANES, 2), np.float32)
        for j in range(NCH):
            s = c * TLOC + j * CSZ
            if c == 0 and j == 0:
                initv[0:32, 0] = fv_init[:32]
                initv[0:32, 1] = fv_init[32:]
            else:
                v = np.float32(vhat(s))
                initv[j * 32:(j + 1) * 32, :] = v
        in_maps.append({
            "videot": shard,
            "wt": wt,
            "bias": bvec[None, :],
            "transt": transt,
            "transf": transf,
            "initv": initv,
        })

    nc = _get_nc("default")
    res = run_bass_kernel_spmd(
        nc, in_maps, list(range(NCORES)), trace=bool(_trace)
    )
    LAST_EXEC_TIME_NS = res.exec_time_ns

    # ---- stitch backpointers ----
    full_bp = np.empty((T, P), np.uint8)
    for c in range(NCORES):
        out_bp = res.results[c]["bptrs"]
        base = 0 if c == 0 else c * TLOC - L0
        for j in range(NCH):
            e0 = 0 if (c == 0 and j == 0) else L0
            t_lo = base + j * CSZ + e0
            t_hi = min(base + j * CSZ + Wn, T)
            full_bp[t_lo:t_hi] = out_bp[j * Wn + e0:j * Wn + e0 + (t_hi - t_lo)]

    # ---- terminal state from last core ----
    fvout = res.results[NCORES - 1]["fvout"]
    fvT = np.empty(P, np.float32)
    fvT[:32] = fvout[(NCH - 1) * 32:NCH * 32, 0]
    fvT[32:] = fvout[(NCH - 1) * 32:NCH * 32, 1]
    terminal = (fvT + trans[STOP]).astype(np.float32)
    best = int(terminal.argmax())

    # ---- backtrace ----
    tags = np.empty(T, np.int64)
    tag = best
    bp_list = full_bp.tolist()
    for t in range(T - 1, -1, -1):
        tags[t] = tag
        tag = bp_list[t][tag]

    # ---- feats (from device) + path score in reference op order ----
    feats = np.empty((T, P), np.float32)
    for c in range(NCORES):
        ft = res.results[c]["featst"]
        off = 0 if c == 0 else L0
        feats[c * TLOC:(c + 1) * TLOC] = ft[:, off:off + TLOC].T

    s_val = np.float32(0.0)
    prev = START
    tr = trans
    for t in range(T):
        tg = int(tags[t])
        s_val = np.float32(np.float32(s_val + tr[tg, prev]) + feats[t, tg])
        prev = tg
    s_val = np.float32(s_val + tr[STOP, int(tags[T - 1])])

    return np.float32(s_val), tags.astype(np.int32)


# revision 36
# speedup vs baseline: 1.1209x; 1.1209x over previous
"""Trainium2 Bass kernel for ActionRecognitionCRF (emission GEMM + Viterbi decode).

Strategy (8 NeuronCores, no collectives):
  - Each core gets an overlapping video shard (its 2048 rows + the previous
    L0=128 rows) transposed on host to [K, rows], computes its feats window
    with the TensorEngine (weights stationary, video streaming, bias via a
    K=1 matmul, ReLU on the Scalar engine) directly in transposed [P, t]
    layout.
  - The Viterbi forward scan is chunk-parallel: each core runs 4 sub-chunk
    scans simultaneously on 4x32 lane groups (states packed 2-per-lane so the
    cross-state max stays inside 32-lane blocks where the DVE stream-transpose
    works).  Sub-chunks (except the global first) start from a host-calibrated
    magnitude-matched uniform init and burn in for L0 steps; max-plus
    coalescence plus grid alignment make the post-burn-in backpointers exact.
  - Backpointers are recomputed from the stored forward vectors in a second,
    fully parallel pass (PE transposes fv history and the scan's max history
    to [t, p] tiles; DVE add + max_index extract first-occurrence argmaxes,
    matching jnp.argmax tie semantics).
  - Host stitches backpointers, runs the O(T) backtrace and recomputes the
    path score along the decoded path in reference op order.
"""

import numpy as np

import concourse.bass as bass
import concourse.tile as tile
from concourse import bacc, mybir
from concourse.bass_utils import run_bass_kernel_spmd
from concourse.masks import make_identity

dt = mybir.dt

P = 64
START = P - 2
STOP = P - 1
NEG = np.float32(-10000.0)

CFG = dict(T=16384, K=4096, NCORES=8, NCH=4, CSZ=512, L0=48)

LAST_EXEC_TIME_NS = None


def _derived(cfg):
    d = dict(cfg)
    d["W"] = cfg["CSZ"] + cfg["L0"]          # scan window length per sub-chunk
    d["TLOC"] = cfg["NCH"] * cfg["CSZ"]      # real steps per core
    d["ROWS"] = d["TLOC"] + cfg["L0"]        # video rows per core
    d["LANES"] = 32 * cfg["NCH"]
    d["KB"] = cfg["K"] // 128
    return d


def build_graph(cfg=CFG):
    d = _derived(cfg)
    K, W, ROWS, LANES, KB, NCH, CSZ, L0 = (
        d["K"], d["W"], d["ROWS"], d["LANES"], d["KB"], d["NCH"], d["CSZ"], d["L0"]
    )
    nc = bacc.Bacc(
        "TRN2", target_bir_lowering=False, debug=False, num_devices=cfg["NCORES"]
    )
    videot_d = nc.dram_tensor("videot", [K, ROWS], dt.float32, kind="ExternalInput").ap()
    wt_d = nc.dram_tensor("wt", [K, P], dt.float32, kind="ExternalInput").ap()
    bias_d = nc.dram_tensor("bias", [1, P], dt.float32, kind="ExternalInput").ap()
    transt_d = nc.dram_tensor("transt", [P, P], dt.float32, kind="ExternalInput").ap()
    transf_d = nc.dram_tensor("transf", [1, P * P], dt.float32, kind="ExternalInput").ap()
    initv_d = nc.dram_tensor("initv", [LANES, 2], dt.float32, kind="ExternalInput").ap()
    bptrs_d = nc.dram_tensor("bptrs", [NCH * W, P], dt.uint8, kind="ExternalOutput").ap()
    fvout_d = nc.dram_tensor("fvout", [LANES, 2], dt.float32, kind="ExternalOutput").ap()
    featst_d = nc.dram_tensor("featst", [P, ROWS], dt.float32, kind="ExternalOutput").ap()

    with tile.TileContext(nc) as tc:
        with (
            tc.tile_pool(name="persist", bufs=1) as pp,
            tc.tile_pool(name="vt", bufs=4) as vpool,
            tc.tile_pool(name="vt0", bufs=8) as vpool0,
            tc.tile_pool(name="scan", bufs=2) as spool,
            tc.tile_pool(name="bptr", bufs=2) as bpool,
            tc.tile_pool(name="psum_f", bufs=2, space="PSUM") as psf,
            tc.tile_pool(name="psum_t", bufs=2, space="PSUM") as pst,
        ):
            # ---- static setup ----
            wts = pp.tile([128, KB, P], dt.float32)
            nc.sync.dma_start(wts[:], wt_d.rearrange("(kb r) p -> r kb p", r=128))
            bias_sb = pp.tile([1, P], dt.float32)
            nc.sync.dma_start(bias_sb[:], bias_d[:])
            ones = pp.tile([1, 512], dt.float32)
            nc.vector.memset(ones[:], 1.0)
            ident = pp.tile([128, 32], dt.float32)
            make_identity(nc, ident[0:32, :])
            for j in range(1, 4):
                nc.sync.dma_start(ident[j * 32:(j + 1) * 32, :], ident[0:32, :])

            transT = pp.tile([LANES, 2, P], dt.float32)
            for j in range(NCH):
                nc.sync.dma_start(
                    transT[j * 32:(j + 1) * 32, :, :],
                    transt_d.rearrange("(pb i) n -> i pb n", i=32),
                )
            transBC = pp.tile([128, P, P], dt.float32)
            nc.sync.dma_start(
                transBC.rearrange("r n p -> r (n p)"),
                transf_d[:].broadcast_to([128, P * P]),
            )

            M = pp.tile([LANES, 2, W + 1], dt.float32)
            nc.sync.dma_start(M[:, :, 0], initv_d[:])
            FEAT2 = pp.tile([LANES, 2, W + 1], dt.float32)
            nc.vector.memset(FEAT2[:, :, 0], 0.0)
            FVH = pp.tile([LANES, 2, W + 1], dt.float32)
            # feats in per-block tiles, GEMM'd in window-interleaved order so
            # the scan (which consumes one column of every window per step)
            # can start after the first round of blocks instead of after the
            # whole GEMM
            GB = min(256, CSZ)
            fb_sizes = []
            for s0 in range(0, CSZ, GB):
                for j in range(NCH):
                    fb_sizes.append((j * CSZ + s0, min(GB, ROWS - (j * CSZ + s0))))
            t0 = NCH * CSZ
            while t0 < ROWS:
                fb_sizes.append((t0, min(GB, ROWS - t0)))
                t0 += min(GB, ROWS - t0)
            fblocks = [
                pp.tile([P, nt], dt.float32, tag=f"fb{i}", name=f"fb{i}")
                for i, (_, nt) in enumerate(fb_sizes)
            ]

            # all (window, half, segment) fills keyed by the global col range
            segs = []
            for j in range(NCH):
                for b in range(2):
                    s0 = 0
                    while s0 < W:
                        ns = min(128, W - s0)
                        segs.append((j, b, s0, ns))
                        s0 += ns

            # ---- GEMM: feats[p, t] = relu(video @ W.T + b).T ----
            # Window fills are emitted right after the block that produces
            # their data so they don't queue behind the whole GEMM.
            for i, (t0, nt) in enumerate(fb_sizes):
                psum = psf.tile([P, 512], dt.float32)
                for kb in range(KB):
                    if i < NCH:
                        vt = vpool0.tile([128, 512], dt.float32, name="vt0")
                    else:
                        vt = vpool.tile([128, 512], dt.float32, name="vt")
                    nc.sync.dma_start(
                        vt[:, 0:nt], videot_d[kb * 128:(kb + 1) * 128, t0:t0 + nt]
                    )
                    nc.tensor.matmul(
                        psum[:, 0:nt], wts[:, kb, :], vt[:, 0:nt],
                        start=(kb == 0), stop=False,
                    )
                nc.tensor.matmul(
                    psum[:, 0:nt], bias_sb[:], ones[:, 0:nt], start=False, stop=True
                )
                nc.scalar.activation(
                    fblocks[i][:], psum[:, 0:nt],
                    mybir.ActivationFunctionType.Relu,
                )
                for (j, b, s0, ns) in segs:
                    lo = j * CSZ + s0
                    if t0 <= lo and lo + ns <= t0 + nt:
                        nc.sync.dma_start(
                            FEAT2[j * 32:(j + 1) * 32, b, 1 + s0:1 + s0 + ns],
                            fblocks[i][32 * b:32 * b + 32, lo - t0:lo - t0 + ns],
                        )

            # ---- Viterbi scan (NCH sub-chunks in parallel on lane groups) ----
            for w in range(W + 1):
                nc.vector.tensor_add(FVH[:, :, w], M[:, :, w], FEAT2[:, :, w])
                if w == W:
                    break
                st = spool.tile([LANES, 2, P], dt.float32)
                nc.vector.tensor_add(
                    st[:], transT[:], FVH[:, :, w:w + 1].broadcast_to([LANES, 2, P])
                )
                x = spool.tile([LANES, 2, 2, 32], dt.float32)  # (pb, nb, k)
                nc.vector.transpose(
                    x.rearrange("p pb nb k -> p (pb nb k)"),
                    st.rearrange("p pb n -> p (pb n)"),
                )
                nc.vector.tensor_reduce(
                    M[:, :, w + 1],
                    x.rearrange("p pb nb k -> p nb pb k"),
                    axis=mybir.AxisListType.XY,
                    op=mybir.AluOpType.max,
                )
            nc.sync.dma_start(fvout_d[:], FVH[:, :, W])

            # ---- backpointer recompute pass ----
            # Reuses the scan's max values (M history) so only the index
            # search and the batched adds run on DVE.
            NB = 8
            for j in range(NCH):
                w0 = 0 if j == 0 else L0
                while w0 < W:
                    nr = min(128, W - w0)
                    pfv = pst.tile([128, P], dt.float32)
                    pmx = pst.tile([128, P], dt.float32)
                    for b in range(2):
                        nc.tensor.transpose(
                            pfv[0:nr, b * 32:(b + 1) * 32],
                            FVH[j * 32:(j + 1) * 32, b, w0:w0 + nr],
                            ident[j * 32:(j + 1) * 32, :],
                            tile_position=(j * 32, 0),
                        )
                        nc.tensor.transpose(
                            pmx[0:nr, b * 32:(b + 1) * 32],
                            M[j * 32:(j + 1) * 32, b, w0 + 1:w0 + 1 + nr],
                            ident[j * 32:(j + 1) * 32, :],
                            tile_position=(j * 32, 0),
                        )
                    fv_sb = bpool.tile([128, P], dt.float32)
                    nc.scalar.activation(
                        fv_sb[0:nr, :], pfv[0:nr, :],
                        mybir.ActivationFunctionType.Copy,
                    )
                    mx_sb = bpool.tile([128, P], dt.float32)
                    nc.scalar.activation(
                        mx_sb[0:nr, :], pmx[0:nr, :],
                        mybir.ActivationFunctionType.Copy,
                    )
                    bp8 = bpool.tile([128, P, 8], dt.uint32)
                    for n0 in range(0, P, NB):
                        tmp = bpool.tile([128, NB, P], dt.float32)
                        nc.vector.tensor_add(
                            tmp[0:nr, :, :],
                            transBC[0:nr, n0:n0 + NB, :],
                            fv_sb[0:nr, :].rearrange(
                                "t (o p) -> t o p", o=1
                            ).broadcast_to([nr, NB, P]),
                        )
                        for k in range(NB):
                            nc.vector.max_index(
                                bp8[0:nr, n0 + k, :],
                                mx_sb[0:nr, n0 + k:n0 + k + 1].broadcast_to([nr, 8]),
                                tmp[0:nr, k, :],
                            )
                    bpu8 = bpool.tile([128, P], dt.uint8)
                    nc.vector.tensor_copy(bpu8[0:nr, :], bp8[0:nr, :, 0])
                    nc.sync.dma_start(
                        bptrs_d[j * W + w0:j * W + w0 + nr, :], bpu8[0:nr, :]
                    )
                    w0 += nr

            for (t0, nt), fb in zip(fb_sizes, fblocks):
                nc.sync.dma_start(featst_d[:, t0:t0 + nt], fb[:])

    nc.compile()
    return nc


_NC_CACHE = {}


def _get_nc(cfg_key):
    if cfg_key not in _NC_CACHE:
        _NC_CACHE[cfg_key] = build_graph(CFG)
    return _NC_CACHE[cfg_key]


def _scan_fv(feats_win, fv, trans):
    fv = fv.copy()
    hist = np.empty((feats_win.shape[0], P), np.float32)
    for t in range(feats_win.shape[0]):
        scores = (fv[None, :] + trans).astype(np.float32)
        fv = (scores.max(axis=1) + feats_win[t]).astype(np.float32)
        hist[t] = fv
    return hist


def _calibrate(video, Wm, bvec, trans, cfg):
    """Estimate the forward-vector magnitude at each sub-chunk start via an
    anchored mini-scan + sampled slopes (sets only scan init constants)."""
    T = cfg["T"]
    fv_init = np.full(P, NEG, np.float32)
    fv_init[START] = 0.0

    def host_feats(sl):
        return np.maximum(
            (video[sl].astype(np.float32) @ Wm.T + bvec).astype(np.float32), 0.0
        )

    h0 = _scan_fv(host_feats(slice(0, 384)), fv_init, trans)
    phi0 = h0.max(axis=1)
    anchor_v = phi0[-1]
    slopes = {384: (phi0[383] - phi0[255]) / 128.0}
    n_anchor = max(1, T // 2048)
    anchor_ts = [384]
    for m in range(1, n_anchor):
        tm = 2048 * m
        hw = _scan_fv(
            host_feats(slice(tm - 96, tm + 160)), np.zeros(P, np.float32), trans
        )
        phi = hw.max(axis=1)
        slopes[tm] = (phi[255] - phi[127]) / 128.0
        anchor_ts.append(tm)

    def vhat(s):
        if s <= 384:
            return anchor_v + slopes[384] * (s - 384)
        v = anchor_v
        prev = 384
        for a in anchor_ts[1:] + [T]:
            seg_end = min(a, s)
            sl_lo = slopes[prev] if prev in slopes else slopes[384]
            sl_hi = slopes[a] if a in slopes else slopes[anchor_ts[-1]]
            v += (sl_lo + sl_hi) / 2 * (seg_end - prev)
            prev = seg_end
            if seg_end >= s:
                break
        return v

    return vhat


def kernel(video, W, b, transitions, _trace=False):
    global LAST_EXEC_TIME_NS
    cfg = CFG
    d = _derived(cfg)
    T, K, NCORES, NCH, CSZ, L0 = (
        cfg["T"], cfg["K"], cfg["NCORES"], cfg["NCH"], cfg["CSZ"], cfg["L0"]
    )
    Wn, TLOC, ROWS, LANES = d["W"], d["TLOC"], d["ROWS"], d["LANES"]

    video = np.ascontiguousarray(np.asarray(video, np.float32))
    Wm = np.ascontiguousarray(np.asarray(W, np.float32))
    bvec = np.asarray(b, np.float32).reshape(-1)
    trans = np.ascontiguousarray(np.asarray(transitions, np.float32))

    fv_init = np.full(P, NEG, np.float32)
    fv_init[START] = 0.0

    vhat = _calibrate(video, Wm, bvec, trans, cfg)

    videot = np.ascontiguousarray(video.T)  # [K, T]
    wt = np.ascontiguousarray(Wm.T)         # [K, P]
    transt = np.ascontiguousarray(trans.T)
    transf = np.ascontiguousarray(trans.reshape(1, P * P))

    in_maps = []
    for c in range(NCORES):
        a = 0 if c == 0 else c * TLOC - L0
        shard = np.ascontiguousarray(videot[:, a:a + ROWS])
        initv = np.empty((LANES, 2), np.float32)
        for j in range(NCH):
            s = c * TLOC + j * CSZ
            if c == 0 and j == 0:
                initv[0:32, 0] = fv_init[:32]
                initv[0:32, 1] = fv_init[32:]
            else:
                v = np.float32(vhat(s))
                initv[j * 32:(j + 1) * 32, :] = v
        in_maps.append({
            "videot": shard,
            "wt": wt,
            "bias": bvec[None, :],
            "transt": transt,
            "transf": transf,
            "initv": initv,
        })

    nc = _get_nc("default")
    res = run_bass_kernel_spmd(
        nc, in_maps, list(range(NCORES)), trace=bool(_trace)
    )
    LAST_EXEC_TIME_NS = res.exec_time_ns

    # ---- stitch backpointers ----
    full_bp = np.empty((T, P), np.uint8)
    for c in range(NCORES):
        out_bp = res.results[c]["bptrs"]
        base = 0 if c == 0 else c * TLOC - L0
        for j in range(NCH):
            e0 = 0 if (c == 0 and j == 0) else L0
            t_lo = base + j * CSZ + e0
            t_hi = min(base + j * CSZ + Wn, T)
            full_bp[t_lo:t_hi] = out_bp[j * Wn + e0:j * Wn + e0 + (t_hi - t_lo)]

    # ---- terminal state from last core ----
    fvout = res.results[NCORES - 1]["fvout"]
    fvT = np.empty(P, np.float32)
    fvT[:32] = fvout[(NCH - 1) * 32:NCH * 32, 0]
    fvT[32:] = fvout[(NCH - 1) * 32:NCH * 32, 1]
    terminal = (fvT + trans[STOP]).astype(np.float32)
    best = int(terminal.argmax())

    # ---- backtrace ----
    tags = np.empty(T, np.int64)
    tag = best
    bp_list = full_bp.tolist()
    for t in range(T - 1, -1, -1):
        tags[t] = tag
        tag = bp_list[t][tag]

    # ---- feats (from device) + path score in reference op order ----
    feats = np.empty((T, P), np.float32)
    for c in range(NCORES):
        ft = res.results[c]["featst"]
        off = 0 if c == 0 else L0
        feats[c * TLOC:(c + 1) * TLOC] = ft[:, off:off + TLOC].T

    s_val = np.float32(0.0)
    prev = START
    tr = trans
    for t in range(T):
        tg = int(tags[t])
        s_val = np.float32(np.float32(s_val + tr[tg, prev]) + feats[t, tg])
        prev = tg
    s_val = np.float32(s_val + tr[STOP, int(tags[T - 1])])

    return np.float32(s_val), tags.astype(np.int32)


# revision 38
# speedup vs baseline: 1.1600x; 1.0349x over previous
"""Trainium2 Bass kernel for ActionRecognitionCRF (emission GEMM + Viterbi decode).

Strategy (8 NeuronCores, no collectives):
  - Each core gets an overlapping video shard (its 2048 rows + the previous
    L0=128 rows) transposed on host to [K, rows], computes its feats window
    with the TensorEngine (weights stationary, video streaming, bias via a
    K=1 matmul, ReLU on the Scalar engine) directly in transposed [P, t]
    layout.
  - The Viterbi forward scan is chunk-parallel: each core runs 4 sub-chunk
    scans simultaneously on 4x32 lane groups (states packed 2-per-lane so the
    cross-state max stays inside 32-lane blocks where the DVE stream-transpose
    works).  Sub-chunks (except the global first) start from a host-calibrated
    magnitude-matched uniform init and burn in for L0 steps; max-plus
    coalescence plus grid alignment make the post-burn-in backpointers exact.
  - Backpointers are recomputed from the stored forward vectors in a second,
    fully parallel pass (PE transposes fv history and the scan's max history
    to [t, p] tiles; DVE add + max_index extract first-occurrence argmaxes,
    matching jnp.argmax tie semantics).
  - Host stitches backpointers, runs the O(T) backtrace and recomputes the
    path score along the decoded path in reference op order.
"""

import numpy as np

import concourse.bass as bass
import concourse.tile as tile
from concourse import bacc, mybir
from concourse.bass_utils import run_bass_kernel_spmd
from concourse.masks import make_identity

dt = mybir.dt

P = 64
START = P - 2
STOP = P - 1
NEG = np.float32(-10000.0)

CFG = dict(T=16384, K=4096, NCORES=8, NCH=4, CSZ=512, L0=32)

LAST_EXEC_TIME_NS = None


def _derived(cfg):
    d = dict(cfg)
    d["W"] = cfg["CSZ"] + cfg["L0"]          # scan window length per sub-chunk
    d["TLOC"] = cfg["NCH"] * cfg["CSZ"]      # real steps per core
    d["ROWS"] = d["TLOC"] + cfg["L0"]        # video rows per core
    d["LANES"] = 32 * cfg["NCH"]
    d["KB"] = cfg["K"] // 128
    return d


def build_graph(cfg=CFG):
    d = _derived(cfg)
    K, W, ROWS, LANES, KB, NCH, CSZ, L0 = (
        d["K"], d["W"], d["ROWS"], d["LANES"], d["KB"], d["NCH"], d["CSZ"], d["L0"]
    )
    nc = bacc.Bacc(
        "TRN2", target_bir_lowering=False, debug=False, num_devices=cfg["NCORES"]
    )
    videot_d = nc.dram_tensor("videot", [K, ROWS], dt.float32, kind="ExternalInput").ap()
    wt_d = nc.dram_tensor("wt", [K, P], dt.float32, kind="ExternalInput").ap()
    bias_d = nc.dram_tensor("bias", [1, P], dt.float32, kind="ExternalInput").ap()
    transt_d = nc.dram_tensor("transt", [P, P], dt.float32, kind="ExternalInput").ap()
    transf_d = nc.dram_tensor("transf", [1, P * P], dt.float32, kind="ExternalInput").ap()
    initv_d = nc.dram_tensor("initv", [LANES, 2], dt.float32, kind="ExternalInput").ap()
    bptrs_d = nc.dram_tensor("bptrs", [NCH * W, P], dt.uint8, kind="ExternalOutput").ap()
    fvout_d = nc.dram_tensor("fvout", [LANES, 2], dt.float32, kind="ExternalOutput").ap()
    featst_d = nc.dram_tensor("featst", [P, ROWS], dt.float32, kind="ExternalOutput").ap()

    with tile.TileContext(nc) as tc:
        with (
            tc.tile_pool(name="persist", bufs=1) as pp,
            tc.tile_pool(name="vt", bufs=4) as vpool,
            tc.tile_pool(name="vt0", bufs=8) as vpool0,
            tc.tile_pool(name="scan", bufs=2) as spool,
            tc.tile_pool(name="bptr", bufs=2) as bpool,
            tc.tile_pool(name="psum_f", bufs=2, space="PSUM") as psf,
            tc.tile_pool(name="psum_t", bufs=2, space="PSUM") as pst,
        ):
            # ---- static setup ----
            wts = pp.tile([128, KB, P], dt.float32)
            nc.sync.dma_start(wts[:], wt_d.rearrange("(kb r) p -> r kb p", r=128))
            bias_sb = pp.tile([1, P], dt.float32)
            nc.sync.dma_start(bias_sb[:], bias_d[:])
            ones = pp.tile([1, 512], dt.float32)
            nc.vector.memset(ones[:], 1.0)
            ident = pp.tile([128, 32], dt.float32)
            make_identity(nc, ident[0:32, :])
            for j in range(1, 4):
                nc.sync.dma_start(ident[j * 32:(j + 1) * 32, :], ident[0:32, :])

            transT = pp.tile([LANES, 2, P], dt.float32)
            for j in range(NCH):
                nc.sync.dma_start(
                    transT[j * 32:(j + 1) * 32, :, :],
                    transt_d.rearrange("(pb i) n -> i pb n", i=32),
                )
            transBC = pp.tile([128, P, P], dt.float32)
            nc.sync.dma_start(
                transBC.rearrange("r n p -> r (n p)"),
                transf_d[:].broadcast_to([128, P * P]),
            )

            M = pp.tile([LANES, 2, W + 1], dt.float32)
            nc.sync.dma_start(M[:, :, 0], initv_d[:])
            FEAT2 = pp.tile([LANES, 2, W + 1], dt.float32)
            nc.vector.memset(FEAT2[:, :, 0], 0.0)
            FVH = pp.tile([LANES, 2, W + 1], dt.float32)
            # feats in per-block tiles, GEMM'd in window-interleaved order so
            # the scan (which consumes one column of every window per step)
            # can start after the first round of blocks instead of after the
            # whole GEMM
            GB = min(256, CSZ)
            fb_sizes = []
            for s0 in range(0, CSZ, GB):
                for j in range(NCH):
                    fb_sizes.append((j * CSZ + s0, min(GB, ROWS - (j * CSZ + s0))))
            t0 = NCH * CSZ
            while t0 < ROWS:
                fb_sizes.append((t0, min(GB, ROWS - t0)))
                t0 += min(GB, ROWS - t0)
            fblocks = [
                pp.tile([P, nt], dt.float32, tag=f"fb{i}", name=f"fb{i}")
                for i, (_, nt) in enumerate(fb_sizes)
            ]

            # all (window, half, segment) fills keyed by the global col range
            segs = []
            for j in range(NCH):
                for b in range(2):
                    s0 = 0
                    while s0 < W:
                        ns = min(128, W - s0)
                        segs.append((j, b, s0, ns))
                        s0 += ns

            # ---- GEMM: feats[p, t] = relu(video @ W.T + b).T ----
            # Window fills are emitted right after the block that produces
            # their data so they don't queue behind the whole GEMM.
            for i, (t0, nt) in enumerate(fb_sizes):
                psum = psf.tile([P, 512], dt.float32)
                for kb in range(KB):
                    if i < NCH:
                        vt = vpool0.tile([128, 512], dt.float32, name="vt0")
                    else:
                        vt = vpool.tile([128, 512], dt.float32, name="vt")
                    nc.sync.dma_start(
                        vt[:, 0:nt], videot_d[kb * 128:(kb + 1) * 128, t0:t0 + nt]
                    )
                    nc.tensor.matmul(
                        psum[:, 0:nt], wts[:, kb, :], vt[:, 0:nt],
                        start=(kb == 0), stop=False,
                    )
                nc.tensor.matmul(
                    psum[:, 0:nt], bias_sb[:], ones[:, 0:nt], start=False, stop=True
                )
                nc.scalar.activation(
                    fblocks[i][:], psum[:, 0:nt],
                    mybir.ActivationFunctionType.Relu,
                )
                for (j, b, s0, ns) in segs:
                    lo = j * CSZ + s0
                    if t0 <= lo and lo + ns <= t0 + nt:
                        nc.sync.dma_start(
                            FEAT2[j * 32:(j + 1) * 32, b, 1 + s0:1 + s0 + ns],
                            fblocks[i][32 * b:32 * b + 32, lo - t0:lo - t0 + ns],
                        )

            # ---- Viterbi scan (NCH sub-chunks in parallel on lane groups) ----
            for w in range(W + 1):
                nc.vector.tensor_add(FVH[:, :, w], M[:, :, w], FEAT2[:, :, w])
                if w == W:
                    break
                st = spool.tile([LANES, 2, P], dt.float32)
                nc.vector.tensor_add(
                    st[:], transT[:], FVH[:, :, w:w + 1].broadcast_to([LANES, 2, P])
                )
                x = spool.tile([LANES, 2, 2, 32], dt.float32)  # (pb, nb, k)
                nc.vector.transpose(
                    x.rearrange("p pb nb k -> p (pb nb k)"),
                    st.rearrange("p pb n -> p (pb n)"),
                )
                nc.vector.tensor_reduce(
                    M[:, :, w + 1],
                    x.rearrange("p pb nb k -> p nb pb k"),
                    axis=mybir.AxisListType.XY,
                    op=mybir.AluOpType.max,
                )
            nc.sync.dma_start(fvout_d[:], FVH[:, :, W])

            # ---- backpointer recompute pass ----
            # Reuses the scan's max values (M history) so only the index
            # search and the batched adds run on DVE.
            NB = 8
            for j in range(NCH):
                w0 = 0 if j == 0 else L0
                while w0 < W:
                    nr = min(128, W - w0)
                    pfv = pst.tile([128, P], dt.float32)
                    pmx = pst.tile([128, P], dt.float32)
                    for b in range(2):
                        nc.tensor.transpose(
                            pfv[0:nr, b * 32:(b + 1) * 32],
                            FVH[j * 32:(j + 1) * 32, b, w0:w0 + nr],
                            ident[j * 32:(j + 1) * 32, :],
                            tile_position=(j * 32, 0),
                        )
                        nc.tensor.transpose(
                            pmx[0:nr, b * 32:(b + 1) * 32],
                            M[j * 32:(j + 1) * 32, b, w0 + 1:w0 + 1 + nr],
                            ident[j * 32:(j + 1) * 32, :],
                            tile_position=(j * 32, 0),
                        )
                    fv_sb = bpool.tile([128, P], dt.float32)
                    nc.scalar.activation(
                        fv_sb[0:nr, :], pfv[0:nr, :],
                        mybir.ActivationFunctionType.Copy,
                    )
                    mx_sb = bpool.tile([128, P], dt.float32)
                    nc.scalar.activation(
                        mx_sb[0:nr, :], pmx[0:nr, :],
                        mybir.ActivationFunctionType.Copy,
                    )
                    bp8 = bpool.tile([128, P, 8], dt.uint32)
                    for n0 in range(0, P, NB):
                        tmp = bpool.tile([128, NB, P], dt.float32)
                        nc.vector.tensor_add(
                            tmp[0:nr, :, :],
                            transBC[0:nr, n0:n0 + NB, :],
                            fv_sb[0:nr, :].rearrange(
                                "t (o p) -> t o p", o=1
                            ).broadcast_to([nr, NB, P]),
                        )
                        for k in range(NB):
                            nc.vector.max_index(
                                bp8[0:nr, n0 + k, :],
                                mx_sb[0:nr, n0 + k:n0 + k + 1].broadcast_to([nr, 8]),
                                tmp[0:nr, k, :],
                            )
                    bpu8 = bpool.tile([128, P], dt.uint8)
                    nc.vector.tensor_copy(bpu8[0:nr, :], bp8[0:nr, :, 0])
                    nc.sync.dma_start(
                        bptrs_d[j * W + w0:j * W + w0 + nr, :], bpu8[0:nr, :]
                    )
                    w0 += nr

            for (t0, nt), fb in zip(fb_sizes, fblocks):
                nc.sync.dma_start(featst_d[:, t0:t0 + nt], fb[:])

    nc.compile()
    return nc


_NC_CACHE = {}


def _get_nc(cfg_key):
    if cfg_key not in _NC_CACHE:
        _NC_CACHE[cfg_key] = build_graph(CFG)
    return _NC_CACHE[cfg_key]


def _scan_fv(feats_win, fv, trans):
    fv = fv.copy()
    hist = np.empty((feats_win.shape[0], P), np.float32)
    for t in range(feats_win.shape[0]):
        scores = (fv[None, :] + trans).astype(np.float32)
        fv = (scores.max(axis=1) + feats_win[t]).astype(np.float32)
        hist[t] = fv
    return hist


def _calibrate(video, Wm, bvec, trans, cfg):
    """Estimate the forward-vector magnitude at each sub-chunk start via an
    anchored mini-scan + sampled slopes (sets only scan init constants)."""
    T = cfg["T"]
    fv_init = np.full(P, NEG, np.float32)
    fv_init[START] = 0.0

    def host_feats(sl):
        return np.maximum(
            (video[sl].astype(np.float32) @ Wm.T + bvec).astype(np.float32), 0.0
        )

    h0 = _scan_fv(host_feats(slice(0, 384)), fv_init, trans)
    phi0 = h0.max(axis=1)
    anchor_v = phi0[-1]
    slopes = {384: (phi0[383] - phi0[255]) / 128.0}
    n_anchor = max(1, T // 2048)
    anchor_ts = [384]
    for m in range(1, n_anchor):
        tm = 2048 * m
        hw = _scan_fv(
            host_feats(slice(tm - 96, tm + 160)), np.zeros(P, np.float32), trans
        )
        phi = hw.max(axis=1)
        slopes[tm] = (phi[255] - phi[127]) / 128.0
        anchor_ts.append(tm)

    def vhat(s):
        if s <= 384:
            return anchor_v + slopes[384] * (s - 384)
        v = anchor_v
        prev = 384
        for a in anchor_ts[1:] + [T]:
            seg_end = min(a, s)
            sl_lo = slopes[prev] if prev in slopes else slopes[384]
            sl_hi = slopes[a] if a in slopes else slopes[anchor_ts[-1]]
            v += (sl_lo + sl_hi) / 2 * (seg_end - prev)
            prev = seg_end
            if seg_end >= s:
                break
        return v

    return vhat


def kernel(video, W, b, transitions, _trace=False):
    global LAST_EXEC_TIME_NS
    cfg = CFG
    d = _derived(cfg)
    T, K, NCORES, NCH, CSZ, L0 = (
        cfg["T"], cfg["K"], cfg["NCORES"], cfg["NCH"], cfg["CSZ"], cfg["L0"]
    )
    Wn, TLOC, ROWS, LANES = d["W"], d["TLOC"], d["ROWS"], d["LANES"]

    video = np.ascontiguousarray(np.asarray(video, np.float32))
    Wm = np.ascontiguousarray(np.asarray(W, np.float32))
    bvec = np.asarray(b, np.float32).reshape(-1)
    trans = np.ascontiguousarray(np.asarray(transitions, np.float32))

    fv_init = np.full(P, NEG, np.float32)
    fv_init[START] = 0.0

    vhat = _calibrate(video, Wm, bvec, trans, cfg)

    videot = np.ascontiguousarray(video.T)  # [K, T]
    wt = np.ascontiguousarray(Wm.T)         # [K, P]
    transt = np.ascontiguousarray(trans.T)
    transf = np.ascontiguousarray(trans.reshape(1, P * P))

    in_maps = []
    for c in range(NCORES):
        a = 0 if c == 0 else c * TLOC - L0
        shard = np.ascontiguousarray(videot[:, a:a + ROWS])
        initv = np.empty((LANES, 2), np.float32)
        for j in range(NCH):
            s = c * TLOC + j * CSZ
            if c == 0 and j == 0:
                initv[0:32, 0] = fv_init[:32]
                initv[0:32, 1] = fv_init[32:]
            else:
                v = np.float32(vhat(s))
                initv[j * 32:(j + 1) * 32, :] = v
        in_maps.append({
            "videot": shard,
            "wt": wt,
            "bias": bvec[None, :],
            "transt": transt,
            "transf": transf,
            "initv": initv,
        })

    nc = _get_nc("default")
    res = run_bass_kernel_spmd(
        nc, in_maps, list(range(NCORES)), trace=bool(_trace)
    )
    LAST_EXEC_TIME_NS = res.exec_time_ns

    # ---- stitch backpointers ----
    full_bp = np.empty((T, P), np.uint8)
    for c in range(NCORES):
        out_bp = res.results[c]["bptrs"]
        base = 0 if c == 0 else c * TLOC - L0
        for j in range(NCH):
            e0 = 0 if (c == 0 and j == 0) else L0
            t_lo = base + j * CSZ + e0
            t_hi = min(base + j * CSZ + Wn, T)
            full_bp[t_lo:t_hi] = out_bp[j * Wn + e0:j * Wn + e0 + (t_hi - t_lo)]

    # ---- terminal state from last core ----
    fvout = res.results[NCORES - 1]["fvout"]
    fvT = np.empty(P, np.float32)
    fvT[:32] = fvout[(NCH - 1) * 32:NCH * 32, 0]
    fvT[32:] = fvout[(NCH - 1) * 32:NCH * 32, 1]
    terminal = (fvT + trans[STOP]).astype(np.float32)
    best = int(terminal.argmax())

    # ---- backtrace ----
    tags = np.empty(T, np.int64)
    tag = best
    bp_list = full_bp.tolist()
    for t in range(T - 1, -1, -1):
        tags[t] = tag
        tag = bp_list[t][tag]

    # ---- feats (from device) + path score in reference op order ----
    feats = np.empty((T, P), np.float32)
    for c in range(NCORES):
        ft = res.results[c]["featst"]
        off = 0 if c == 0 else L0
        feats[c * TLOC:(c + 1) * TLOC] = ft[:, off:off + TLOC].T

    s_val = np.float32(0.0)
    prev = START
    tr = trans
    for t in range(T):
        tg = int(tags[t])
        s_val = np.float32(np.float32(s_val + tr[tg, prev]) + feats[t, tg])
        prev = tg
    s_val = np.float32(s_val + tr[STOP, int(tags[T - 1])])

    return np.float32(s_val), tags.astype(np.int32)
